# revision 19
# baseline (speedup 1.0000x reference)
"""Bilateral filter (nn_BilateralFilter) Trainium2 Bass kernel.

Semantics (KERNEL_SIZE=5, THETA_ALPHA=2.0, THETA_BETA=0.1):
    w_k   = exp(-(dx^2+dy^2)/8)                      (24 offsets, center dropped)
    Ki    = exp(-50*(I(p+k) - I(p))^2)               per image channel c
    out[c,n,p] = sum_k w_k*Ki[c,k,p]*Q(n,p+k) / sum_k w_k*Ki[c,k,p]

Sharding: 8 cores = 2 batches x 4 col-slabs of 80 output cols.  Per core,
partitions = 128 = (xh in {0,1} col-half of 40) x (row mod 64); free dims =
(row-chunk q in [0,5), channel, x).

v3: every k-fold lives on the Tensor engine via identity-stationary matmuls
accumulating into PSUM:
  - acc  (sum_k kw*Q, 3600 fp32) in PSUM cols [0,3600), 512-col bank chunks
  - norm (sum_k kw) first 496 of its 600 cols in the PSUM bank-7 hole
    [3600,4096); the 104-col tail is a 6-op DVE tree (PSUM is 104 cols short)
DVE does subs, the 24 products, the tiny norm tail, reciprocal and the
final division (read straight from PSUM at 1x, split 1920+1680 so the first
output DMA overlaps the second division).  ACT does Square/Exp and the
fp32 stitch copies.  Outputs go to two contiguous DRAM tensors so the DMA
descriptors coalesce (a strided SBUF->HBM DMA is ~5x slower).
"""

import math

import numpy as np

B, C, NCL = 2, 3, 6
H = W = 320
KS, PAD = 5, 2
SHIFT = 8.0
COEF = 50.0
XSL = W // 4              # 80 output cols per core slab
XWO = 40                  # output cols per half
XWI = XWO + 2 * PAD       # 44 input cols per half
NQ = 5                    # row chunks of 64
PR = 128
HP = H + 2 * PAD          # 324 padded rows

IW = NQ * XWI             # 220   Ia per (s,c)
FW_IA = KS * C * IW       # 3300
QB = NCL * XWI            # 264   Qa per (s,c,q)
FW_QA = KS * C * NQ * QB  # 19800 (Q replicated x3 over c)
SLW = C * NQ * XWO        # 600   d/kw per slot (c,q,x)
NWX = NQ * NCL * XWO      # 1200  per-c product block (q,n,x)
CQN = C * NWX             # 3600  per-j product block (c,q,n,x)
FW_D = KS * KS * SLW      # 15000
FW_N = C * NQ * XWO       # 600   norm (c,q,x)

NPS = 496                 # norm cols accumulated in PSUM (bank-7 hole)
NTL = SLW - NPS           # 104   norm tail cols folded on DVE
CQ_A = 8                  # (c,q) blocks in the first div/DMA half
W_A = CQ_A * NCL * XWO    # 1920
W_B = CQN - W_A           # 1680

# PSUM bank = 512 fp32 per partition; acc chunks must stay inside one bank
MM_CHUNKS = [(j * 512, min((j + 1) * 512, CQN)) for j in range((CQN + 511) // 512)]

_CACHE: dict = {}


def _emit(tc, i_ap, q_ap, oa_ap, ob_ap):
    import concourse.bass as bass
    import concourse.mybir as mybir
    from concourse.masks import make_identity

    f16 = mybir.dt.float16
    f32 = mybir.dt.float32
    AF = mybir.ActivationFunctionType
    nc = tc.nc

    wy = [math.exp(-((s - PAD) ** 2) / 8.0) for s in range(KS)]

    def ap(t, off, dims):
        return bass.AP(tensor=t.tensor, offset=t.offset + off, ap=[[t.shape[1], PR]] + dims)

    with (
        tc.tile_pool(name="p", bufs=1) as pool,
        tc.tile_pool(name="p5p", bufs=2) as p5p,
        tc.tile_pool(name="ps", bufs=1, space="PSUM") as psp,
    ):
        Ia = pool.tile([PR, FW_IA], f16, tag="Ia")
        Qa = pool.tile([PR, FW_QA], f16, tag="Qa")
        d = pool.tile([PR, FW_D], f16, tag="d")
        kw = pool.tile([PR, FW_D], f16, tag="kw")
        ot = pool.tile([PR, CQN], f16, tag="out")

        normT = pool.tile([PR, 2 * NTL], f16, tag="normT")
        nt12 = pool.tile([PR, 12 * NTL], f16, tag="nt12")
        n32 = pool.tile([PR, FW_N], f32, tag="n32")
        r32 = pool.tile([PR, FW_N], f32, tag="r32")

        ident = pool.tile([PR, PR], f16, tag="ident")
        acc = psp.tile([PR, 4096], f32, tag="acc")

        # per-slot exp biases SHIFT + ln(w_k) as const columns (5 distinct)
        bias_vals = sorted(
            {
                SHIFT + math.log(wy[s] * wy[dc])
                for s in range(KS)
                for dc in range(KS)
                if not (s == PAD and dc == PAD)
            }
        )
        bcol = {v: j for j, v in enumerate(bias_vals)}
        bias_t = pool.tile([PR, len(bias_vals)], f32, tag="bias")
        for v, j in bcol.items():
            nc.gpsimd.memset(bias_t[:, j : j + 1], v)

        # center slot of kw zeroed so the norm folds can include it blindly
        nc.gpsimd.memset(kw[:, 12 * SLW : 13 * SLW], 0.0)

        make_identity(nc, ident[:, :])

        # ---- input DMAs: host pre-gathers SBUF layouts; per-s slices on
        # separate queues so the early pipeline blocks land in parallel ----
        S_ORDER = [2, 0, 1, 3, 4]
        ia_q = {2: nc.sync, 0: nc.scalar, 1: nc.scalar, 3: nc.sync, 4: nc.gpsimd}
        for s in S_ORDER:
            ia_q[s].dma_start(
                Ia[:, s * C * IW : (s + 1) * C * IW],
                i_ap[:, s * C * IW : (s + 1) * C * IW],
            )
        QSW = C * NQ * QB  # 3960 per s
        for s in S_ORDER:
            nc.scalar.dma_start(
                Qa[:, s * QSW : (s + 1) * QSW],
                q_ap[:, s * QSW : (s + 1) * QSW],
            )

        # PE p-state warmup during the input-DMA window (bank-0 region is
        # reset by the real chunk-0 group's start=True later).
        for _ in range(12):
            nc.tensor.matmul(
                acc[:, 0:PR], ident[:, :], ident[:, :], start=True, stop=True
            )

        def sub_op(s, dc0, ndc):
            # d[(s,dc), (c,q), x] = Ia_s[(c,q), x+dc] - Ia_2[(c,q), x+2]
            nc.vector.tensor_sub(
                ap(d, (s * KS + dc0) * SLW, [[SLW, ndc], [XWO, C * NQ], [1, XWO]]),
                ap(Ia, s * C * IW + dc0, [[1, ndc], [XWI, C * NQ], [1, XWO]]),
                ap(Ia, 2 * C * IW + PAD, [[0, ndc], [XWI, C * NQ], [1, XWO]]),
            )

        n_slots = 0   # fold-group index over the 24 non-center slots
        n_norm = 0    # norm-group index over all 25 slots

        # Bank 7 ([3584,4096): acc chunk 7 + norm region) is ONE accumulation
        # group: start=True zeroes a whole 2KB bank, so the first bank-7 write
        # (norm_mm of the first slot) starts it and the last fold chunk-7
        # stops it.  Other banks group per-chunk as usual.
        def fold(p5t, off, idx):
            for c0, c1 in MM_CHUNKS:
                in_b7 = c0 >= 3584
                nc.tensor.matmul(
                    acc[:, c0:c1],
                    ident[:, :],
                    p5t[:, off + c0 : off + c1],
                    start=(idx == 0) and not in_b7,
                    stop=(idx == 23),
                )

        def norm_mm(slot, idx):
            nc.tensor.matmul(
                acc[:, 3600 : 3600 + NPS],
                ident[:, :],
                kw[:, slot * SLW : slot * SLW + NPS],
                start=(idx == 0),
                stop=False,
            )

        # ---- per-s pipeline: sub -> square -> exp(+norm mm) -> product ->
        #      PE fold ----
        for si, s in enumerate(S_ORDER):
            if s == PAD:
                sub_op(s, 0, 2)
                sub_op(s, 3, 2)
                nc.scalar.activation(
                    kw[:, (s * KS) * SLW : (s * KS + 2) * SLW],
                    d[:, (s * KS) * SLW : (s * KS + 2) * SLW],
                    AF.Square,
                )
                nc.scalar.activation(
                    kw[:, (s * KS + 3) * SLW : (s * KS + 5) * SLW],
                    d[:, (s * KS + 3) * SLW : (s * KS + 5) * SLW],
                    AF.Square,
                )
            else:
                sub_op(s, 0, KS)
                nc.scalar.activation(
                    kw[:, (s * KS) * SLW : (s * KS + KS) * SLW],
                    d[:, (s * KS) * SLW : (s * KS + KS) * SLW],
                    AF.Square,
                )
            for dc in range(KS):
                slot = s * KS + dc
                if slot != 12:
                    j = bcol[SHIFT + math.log(wy[s] * wy[dc])]
                    nc.scalar.activation(
                        kw[:, slot * SLW : (slot + 1) * SLW],
                        kw[:, slot * SLW : (slot + 1) * SLW],
                        AF.Exp,
                        bias=bias_t[:, j : j + 1],
                        scale=-COEF,
                    )
                norm_mm(slot, n_norm)
                n_norm += 1

            def product_pair(dc):
                # slots (s*5+dc, s*5+dc+1) in one 4-free-dim DVE op (still 2x)
                nonlocal n_slots
                slot = s * KS + dc
                p5t = p5p.tile([PR, 2 * CQN], f16, tag="p5pair")
                nc.vector.tensor_mul(
                    ap(p5t, 0, [[CQN, 2], [NWX // NQ, C * NQ], [XWO, NCL], [1, XWO]]),
                    ap(kw, slot * SLW, [[SLW, 2], [XWO, C * NQ], [0, NCL], [1, XWO]]),
                    ap(Qa, s * QSW + dc, [[1, 2], [QB, C * NQ], [XWI, NCL], [1, XWO]]),
                )
                fold(p5t, 0, n_slots)
                fold(p5t, CQN, n_slots + 1)
                n_slots += 2

            def product_one(dc):
                nonlocal n_slots
                slot = s * KS + dc
                p5t = p5p.tile([PR, CQN], f16, tag="p5")
                nc.vector.tensor_mul(
                    ap(p5t, 0, [[NWX // NQ, C * NQ], [XWO, NCL], [1, XWO]]),
                    ap(kw, slot * SLW, [[XWO, C * NQ], [0, NCL], [1, XWO]]),
                    ap(Qa, s * QSW + dc, [[QB, C * NQ], [XWI, NCL], [1, XWO]]),
                )
                fold(p5t, 0, n_slots)
                n_slots += 1

            product_pair(0)
            if si == 4:
                    # norm 104-col tail: DVE tree over all 25 slots
                    V = nc.vector
                    V.tensor_add(
                        nt12[:, :],
                        ap(kw, NPS, [[SLW, 12], [1, NTL]]),
                        ap(kw, 12 * SLW + NPS, [[SLW, 12], [1, NTL]]),
                    )
                    V.tensor_add(
                        nt12[:, : 6 * NTL], nt12[:, : 6 * NTL], nt12[:, 6 * NTL :]
                    )
                    V.tensor_add(
                        nt12[:, : 3 * NTL],
                        nt12[:, : 3 * NTL],
                        nt12[:, 3 * NTL : 6 * NTL],
                    )
                    V.tensor_add(
                        normT[:, :NTL], nt12[:, :NTL], nt12[:, NTL : 2 * NTL]
                    )
                    V.tensor_add(
                        normT[:, NTL:], normT[:, :NTL], nt12[:, 2 * NTL : 3 * NTL]
                    )
                    V.tensor_add(
                        normT[:, :NTL],
                        normT[:, NTL:],
                        kw[:, 24 * SLW + NPS : 25 * SLW],
                    )
                    nc.scalar.activation(
                        n32[:, NPS:SLW], normT[:, :NTL], AF.Copy
                    )
            if s != PAD:
                product_one(2)
            product_pair(3)

        # ---- tail: drain norm PSUM, reciprocal, split division + DMAs ----
        nc.scalar.activation(n32[:, :NPS], acc[:, 3600 : 3600 + NPS], AF.Copy)
        nc.vector.reciprocal_approx_fast(r32[:, :], n32[:, :])

        CQ = C * NQ  # 15
        nc.vector.tensor_mul(
            ap(ot, 0, [[NCL * XWO, CQ_A], [XWO, NCL], [1, XWO]]),
            ap(acc, 0, [[NCL * XWO, CQ_A], [XWO, NCL], [1, XWO]]),
            ap(r32, 0, [[XWO, CQ_A], [0, NCL], [1, XWO]]),
        )
        nc.scalar.dma_start(oa_ap[:, :], ot[:, :W_A])
        nc.vector.tensor_mul(
            ap(ot, W_A, [[NCL * XWO, CQ - CQ_A], [XWO, NCL], [1, XWO]]),
            ap(acc, W_A, [[NCL * XWO, CQ - CQ_A], [XWO, NCL], [1, XWO]]),
            ap(r32, CQ_A * XWO, [[XWO, CQ - CQ_A], [0, NCL], [1, XWO]]),
        )
        nc.sync.dma_start(ob_ap[:, :], ot[:, W_A:])


def _build_program():
    import concourse.bacc as bacc
    import concourse.mybir as mybir
    from concourse import tile

    f16 = mybir.dt.float16

    nc = bacc.Bacc("TRN2", num_devices=8, debug=False)
    I_in = nc.dram_tensor("i_in", [PR, FW_IA], f16, kind="ExternalInput")
    Q_in = nc.dram_tensor("q_in", [PR, FW_QA], f16, kind="ExternalInput")
    OUT_A = nc.dram_tensor("out_a", [PR, W_A], f16, kind="ExternalOutput")
    OUT_B = nc.dram_tensor("out_b", [PR, W_B], f16, kind="ExternalOutput")

    with tile.TileContext(nc) as tc:
        _emit(tc, I_in.ap(), Q_in.ap(), OUT_A.ap(), OUT_B.ap())

    nc.compile()
    return nc


def _get_program():
    if "nc" not in _CACHE:
        _CACHE["nc"] = _build_program()
    return _CACHE["nc"]


def _gather_i(Xp_sl):
    """(C, 324, 84) padded slab -> (128, (s,c,q,xi44)) fp16."""
    t = np.stack([Xp_sl[:, s : s + H, :] for s in range(KS)])  # (s,C,320,84)
    t = t.reshape(KS, C, NQ, 64, 84)
    t = np.stack([t[..., 40 * xh : 40 * xh + XWI] for xh in range(2)])
    # (xh, s, c, q, rr, xi) -> (xh, rr, s, c, q, xi)
    t = t.transpose(0, 4, 1, 2, 3, 5)
    return np.ascontiguousarray(t.reshape(PR, FW_IA))


def _gather_q(Qp_sl):
    """(NCL, 324, 84) padded slab -> (128, (s,c,q,n,xi44)) fp16, c-replicated."""
    t = np.stack([Qp_sl[:, s : s + H, :] for s in range(KS)])  # (s,NCL,320,84)
    t = t.reshape(KS, NCL, NQ, 64, 84)
    t = np.stack([t[..., 40 * xh : 40 * xh + XWI] for xh in range(2)])
    # (xh, s, n, q, rr, xi) -> (xh, rr, s, q, n, xi)
    t = t.transpose(0, 4, 1, 3, 2, 5)  # (xh, rr, s, q, n, xi)
    t = t.reshape(2, 64, KS, 1, NQ, NCL, XWI)
    t = np.broadcast_to(t, (2, 64, KS, C, NQ, NCL, XWI))
    return np.ascontiguousarray(t.reshape(PR, FW_QA))


def _shard_inputs(Q, I):
    Qp = np.pad(
        np.asarray(Q, np.float32), ((0, 0), (0, 0), (PAD, PAD), (PAD, PAD))
    ).astype(np.float16)
    Ip = np.pad(
        np.asarray(I, np.float32), ((0, 0), (0, 0), (PAD, PAD), (PAD, PAD))
    ).astype(np.float16)
    in_maps = []
    for b in range(B):
        for xs in range(4):
            c0 = xs * XSL
            in_maps.append(
                {
                    "i_in": _gather_i(Ip[b, :, :, c0 : c0 + 84]),
                    "q_in": _gather_q(Qp[b, :, :, c0 : c0 + 84]),
                }
            )
    return in_maps


def _assemble(outs):
    # outs: 8 arrays (128, 3600 = (c,q,n,x)), core order = (b, xs)
    o = np.stack([np.asarray(x) for x in outs]).astype(np.float32)
    o = o.reshape(B, 4, 2, 64, C, NQ, NCL, XWO)
    # (b, xs, xh, rr, c, q, n, x) -> (b, c, n, row=(q,rr), col=(xs,xh,x))
    o = o.transpose(0, 4, 6, 5, 3, 1, 2, 7).reshape(B, C, NCL, H, W)
    return o


def run(Q, I, trace=False):
    from concourse.bass_utils import run_bass_kernel_spmd

    nc = _get_program()
    in_maps = _shard_inputs(Q, I)
    res = run_bass_kernel_spmd(nc, in_maps, list(range(8)), trace=trace)
    out = _assemble(
        [
            np.concatenate(
                [res.results[i]["out_a"], res.results[i]["out_b"]], axis=1
            )
            for i in range(8)
        ]
    )
    return out, res


def kernel(Q, I):
    out, _ = run(Q, I)
    return out


# revision 20
# speedup vs baseline: 1.2551x; 1.2551x over previous
"""Bilateral filter (nn_BilateralFilter) Trainium2 Bass kernel.

Semantics (KERNEL_SIZE=5, THETA_ALPHA=2.0, THETA_BETA=0.1):
    w_k   = exp(-(dx^2+dy^2)/8)                      (24 offsets, center dropped)
    Ki    = exp(-50*(I(p+k) - I(p))^2)               per image channel c
    out[c,n,p] = sum_k w_k*Ki[c,k,p]*Q(n,p+k) / sum_k w_k*Ki[c,k,p]

Sharding: 8 cores = 2 batches x 4 col-slabs of 80 output cols.  Per core,
partitions = 128 = (xh in {0,1} col-half of 40) x (row mod 64); free dims =
(row-chunk q in [0,5), channel, x).

v3: every k-fold lives on the Tensor engine via identity-stationary matmuls
accumulating into PSUM:
  - acc  (sum_k kw*Q, 3600 fp32) in PSUM cols [0,3600), 512-col bank chunks
  - norm (sum_k kw) first 496 of its 600 cols in the PSUM bank-7 hole
    [3600,4096); the 104-col tail is a 6-op DVE tree (PSUM is 104 cols short)
DVE does subs, the 24 products, the tiny norm tail, reciprocal and the
final division (read straight from PSUM at 1x, split 1920+1680 so the first
output DMA overlaps the second division).  ACT does Square/Exp and the
fp32 stitch copies.  Outputs go to two contiguous DRAM tensors so the DMA
descriptors coalesce (a strided SBUF->HBM DMA is ~5x slower).
"""

import math

import numpy as np

B, C, NCL = 2, 3, 6
H = W = 320
KS, PAD = 5, 2
SHIFT = 8.0
COEF = 50.0
XSL = W // 4              # 80 output cols per core slab
XWO = 40                  # output cols per half
XWI = XWO + 2 * PAD       # 44 input cols per half
NQ = 5                    # row chunks of 64
PR = 128
HP = H + 2 * PAD          # 324 padded rows

IW = NQ * XWI             # 220   Ia per (s,c)
FW_IA = KS * C * IW       # 3300
QB = NCL * XWI            # 264   Qa per (s,c,q)
FW_QA = KS * C * NQ * QB  # 19800 (Q replicated x3 over c)
SLW = C * NQ * XWO        # 600   d/kw per slot (c,q,x)
NWX = NQ * NCL * XWO      # 1200  per-c product block (q,n,x)
CQN = C * NWX             # 3600  per-j product block (c,q,n,x)
FW_D = KS * KS * SLW      # 15000
FW_N = C * NQ * XWO       # 600   norm (c,q,x)

NPS = 496                 # norm cols accumulated in PSUM (bank-7 hole)
NTL = SLW - NPS           # 104   norm tail cols folded on DVE
CQ_A = 8                  # (c,q) blocks in the first div/DMA half
W_A = CQ_A * NCL * XWO    # 1920
W_B = CQN - W_A           # 1680

# PSUM bank = 512 fp32 per partition; acc chunks must stay inside one bank
MM_CHUNKS = [(j * 512, min((j + 1) * 512, CQN)) for j in range((CQN + 511) // 512)]

_CACHE: dict = {}


def _emit(tc, i_ap, q_ap, oa_ap, ob_ap):
    import concourse.bass as bass
    import concourse.mybir as mybir
    from concourse.masks import make_identity

    f16 = mybir.dt.float16
    f32 = mybir.dt.float32
    AF = mybir.ActivationFunctionType
    nc = tc.nc

    wy = [math.exp(-((s - PAD) ** 2) / 8.0) for s in range(KS)]

    def ap(t, off, dims):
        return bass.AP(tensor=t.tensor, offset=t.offset + off, ap=[[t.shape[1], PR]] + dims)

    with (
        tc.tile_pool(name="p", bufs=1) as pool,
        tc.tile_pool(name="p5p", bufs=4) as p5p,
        tc.tile_pool(name="ps", bufs=1, space="PSUM") as psp,
    ):
        Ia = pool.tile([PR, FW_IA], f16, tag="Ia")
        Qa = pool.tile([PR, FW_QA], f16, tag="Qa")
        d = pool.tile([PR, FW_D], f16, tag="d")
        kw = pool.tile([PR, FW_D], f16, tag="kw")
        ot = pool.tile([PR, CQN], f16, tag="out")

        normT = pool.tile([PR, 2 * NTL], f16, tag="normT")
        nt12 = pool.tile([PR, 12 * NTL], f16, tag="nt12")
        n32 = pool.tile([PR, FW_N], f32, tag="n32")
        r32 = pool.tile([PR, FW_N], f32, tag="r32")

        ident = pool.tile([PR, PR], f16, tag="ident")
        acc = psp.tile([PR, 4096], f32, tag="acc")

        # per-slot exp biases SHIFT + ln(w_k) as const columns (5 distinct)
        bias_vals = sorted(
            {
                SHIFT + math.log(wy[s] * wy[dc])
                for s in range(KS)
                for dc in range(KS)
                if not (s == PAD and dc == PAD)
            }
        )
        bcol = {v: j for j, v in enumerate(bias_vals)}
        bias_t = pool.tile([PR, len(bias_vals)], f32, tag="bias")
        for v, j in bcol.items():
            nc.gpsimd.memset(bias_t[:, j : j + 1], v)

        # center slot of kw zeroed so the norm folds can include it blindly
        nc.gpsimd.memset(kw[:, 12 * SLW : 13 * SLW], 0.0)

        make_identity(nc, ident[:, :])

        # ---- input DMAs: host pre-gathers SBUF layouts; per-s slices on
        # separate queues so the early pipeline blocks land in parallel ----
        S_ORDER = [2, 0, 1, 3, 4]
        ia_q = {2: nc.sync, 0: nc.scalar, 1: nc.scalar, 3: nc.sync, 4: nc.gpsimd}
        for s in S_ORDER:
            ia_q[s].dma_start(
                Ia[:, s * C * IW : (s + 1) * C * IW],
                i_ap[:, s * C * IW : (s + 1) * C * IW],
            )
        QSW = C * NQ * QB  # 3960 per s
        for s in S_ORDER:
            nc.scalar.dma_start(
                Qa[:, s * QSW : (s + 1) * QSW],
                q_ap[:, s * QSW : (s + 1) * QSW],
            )

        # PE p-state warmup during the input-DMA window (bank-0 region is
        # reset by the real chunk-0 group's start=True later).
        for _ in range(12):
            nc.tensor.matmul(
                acc[:, 0:PR], ident[:, :], ident[:, :], start=True, stop=True
            )

        def sub_op(s, dc0, ndc):
            # d[(s,dc), (c,q), x] = Ia_s[(c,q), x+dc] - Ia_2[(c,q), x+2]
            nc.vector.tensor_sub(
                ap(d, (s * KS + dc0) * SLW, [[SLW, ndc], [XWO, C * NQ], [1, XWO]]),
                ap(Ia, s * C * IW + dc0, [[1, ndc], [XWI, C * NQ], [1, XWO]]),
                ap(Ia, 2 * C * IW + PAD, [[0, ndc], [XWI, C * NQ], [1, XWO]]),
            )

        n_slots = 0   # fold-group index over the 24 non-center slots
        n_norm = 0    # norm-group index over all 25 slots

        # Bank 7 ([3584,4096): acc chunk 7 + norm region) is ONE accumulation
        # group: start=True zeroes a whole 2KB bank, so the first bank-7 write
        # (norm_mm of the first slot) starts it and the last fold chunk-7
        # stops it.  Other banks group per-chunk as usual.
        def fold(p5t, idx):
            for c0, c1 in MM_CHUNKS:
                in_b7 = c0 >= 3584
                nc.tensor.matmul(
                    acc[:, c0:c1],
                    ident[:, :],
                    p5t[:, c0:c1],
                    start=(idx == 0) and not in_b7,
                    stop=(idx == 23),
                )

        def norm_mm(slot, idx):
            nc.tensor.matmul(
                acc[:, 3600 : 3600 + NPS],
                ident[:, :],
                kw[:, slot * SLW : slot * SLW + NPS],
                start=(idx == 0),
                stop=False,
            )

        # ---- per-s pipeline: sub -> square -> exp(+norm mm) -> product ->
        #      PE fold ----
        for si, s in enumerate(S_ORDER):
            if s == PAD:
                sub_op(s, 0, 2)
                sub_op(s, 3, 2)
                nc.scalar.activation(
                    kw[:, (s * KS) * SLW : (s * KS + 2) * SLW],
                    d[:, (s * KS) * SLW : (s * KS + 2) * SLW],
                    AF.Square,
                )
                nc.scalar.activation(
                    kw[:, (s * KS + 3) * SLW : (s * KS + 5) * SLW],
                    d[:, (s * KS + 3) * SLW : (s * KS + 5) * SLW],
                    AF.Square,
                )
            else:
                sub_op(s, 0, KS)
                nc.scalar.activation(
                    kw[:, (s * KS) * SLW : (s * KS + KS) * SLW],
                    d[:, (s * KS) * SLW : (s * KS + KS) * SLW],
                    AF.Square,
                )
            for dc in range(KS):
                slot = s * KS + dc
                if slot != 12:
                    j = bcol[SHIFT + math.log(wy[s] * wy[dc])]
                    nc.scalar.activation(
                        kw[:, slot * SLW : (slot + 1) * SLW],
                        kw[:, slot * SLW : (slot + 1) * SLW],
                        AF.Exp,
                        bias=bias_t[:, j : j + 1],
                        scale=-COEF,
                    )
                norm_mm(slot, n_norm)
                n_norm += 1

            def product(dc):
                slot = s * KS + dc
                p5t = p5p.tile([PR, CQN], f16, tag="p5")
                nc.vector.tensor_mul(
                    ap(p5t, 0, [[NWX // NQ, C * NQ], [XWO, NCL], [1, XWO]]),
                    ap(kw, slot * SLW, [[XWO, C * NQ], [0, NCL], [1, XWO]]),
                    ap(Qa, s * QSW + dc, [[QB, C * NQ], [XWI, NCL], [1, XWO]]),
                )
                return p5t

            dcs = [0, 1, 3, 4] if s == PAD else list(range(KS))
            for i, dc in enumerate(dcs):
                p5t = product(dc)
                fold(p5t, n_slots)
                n_slots += 1
                if si == 4 and i == 1:
                    # norm 104-col tail: DVE tree over all 25 slots
                    V = nc.vector
                    V.tensor_add(
                        nt12[:, :],
                        ap(kw, NPS, [[SLW, 12], [1, NTL]]),
                        ap(kw, 12 * SLW + NPS, [[SLW, 12], [1, NTL]]),
                    )
                    V.tensor_add(
                        nt12[:, : 6 * NTL], nt12[:, : 6 * NTL], nt12[:, 6 * NTL :]
                    )
                    V.tensor_add(
                        nt12[:, : 3 * NTL],
                        nt12[:, : 3 * NTL],
                        nt12[:, 3 * NTL : 6 * NTL],
                    )
                    V.tensor_add(
                        normT[:, :NTL], nt12[:, :NTL], nt12[:, NTL : 2 * NTL]
                    )
                    V.tensor_add(
                        normT[:, NTL:], normT[:, :NTL], nt12[:, 2 * NTL : 3 * NTL]
                    )
                    V.tensor_add(
                        normT[:, :NTL],
                        normT[:, NTL:],
                        kw[:, 24 * SLW + NPS : 25 * SLW],
                    )
                    nc.scalar.activation(
                        n32[:, NPS:SLW], normT[:, :NTL], AF.Copy
                    )

        # ---- tail: drain norm PSUM, reciprocal, split division + DMAs ----
        nc.scalar.activation(n32[:, :NPS], acc[:, 3600 : 3600 + NPS], AF.Copy)
        nc.vector.reciprocal_approx_fast(r32[:, :], n32[:, :])

        CQ = C * NQ  # 15
        nc.vector.tensor_mul(
            ap(ot, 0, [[NCL * XWO, CQ_A], [XWO, NCL], [1, XWO]]),
            ap(acc, 0, [[NCL * XWO, CQ_A], [XWO, NCL], [1, XWO]]),
            ap(r32, 0, [[XWO, CQ_A], [0, NCL], [1, XWO]]),
        )
        nc.scalar.dma_start(oa_ap[:, :], ot[:, :W_A])
        nc.vector.tensor_mul(
            ap(ot, W_A, [[NCL * XWO, CQ - CQ_A], [XWO, NCL], [1, XWO]]),
            ap(acc, W_A, [[NCL * XWO, CQ - CQ_A], [XWO, NCL], [1, XWO]]),
            ap(r32, CQ_A * XWO, [[XWO, CQ - CQ_A], [0, NCL], [1, XWO]]),
        )
        nc.sync.dma_start(ob_ap[:, :], ot[:, W_A:])


def _build_program():
    import concourse.bacc as bacc
    import concourse.mybir as mybir
    from concourse import tile

    f16 = mybir.dt.float16

    nc = bacc.Bacc("TRN2", num_devices=8, debug=False)
    I_in = nc.dram_tensor("i_in", [PR, FW_IA], f16, kind="ExternalInput")
    Q_in = nc.dram_tensor("q_in", [PR, FW_QA], f16, kind="ExternalInput")
    OUT_A = nc.dram_tensor("out_a", [PR, W_A], f16, kind="ExternalOutput")
    OUT_B = nc.dram_tensor("out_b", [PR, W_B], f16, kind="ExternalOutput")

    with tile.TileContext(nc) as tc:
        _emit(tc, I_in.ap(), Q_in.ap(), OUT_A.ap(), OUT_B.ap())

    nc.compile()
    return nc


def _get_program():
    if "nc" not in _CACHE:
        _CACHE["nc"] = _build_program()
    return _CACHE["nc"]


def _gather_i(Xp_sl):
    """(C, 324, 84) padded slab -> (128, (s,c,q,xi44)) fp16."""
    t = np.stack([Xp_sl[:, s : s + H, :] for s in range(KS)])  # (s,C,320,84)
    t = t.reshape(KS, C, NQ, 64, 84)
    t = np.stack([t[..., 40 * xh : 40 * xh + XWI] for xh in range(2)])
    # (xh, s, c, q, rr, xi) -> (xh, rr, s, c, q, xi)
    t = t.transpose(0, 4, 1, 2, 3, 5)
    return np.ascontiguousarray(t.reshape(PR, FW_IA))


def _gather_q(Qp_sl):
    """(NCL, 324, 84) padded slab -> (128, (s,c,q,n,xi44)) fp16, c-replicated."""
    t = np.stack([Qp_sl[:, s : s + H, :] for s in range(KS)])  # (s,NCL,320,84)
    t = t.reshape(KS, NCL, NQ, 64, 84)
    t = np.stack([t[..., 40 * xh : 40 * xh + XWI] for xh in range(2)])
    # (xh, s, n, q, rr, xi) -> (xh, rr, s, q, n, xi)
    t = t.transpose(0, 4, 1, 3, 2, 5)  # (xh, rr, s, q, n, xi)
    t = t.reshape(2, 64, KS, 1, NQ, NCL, XWI)
    t = np.broadcast_to(t, (2, 64, KS, C, NQ, NCL, XWI))
    return np.ascontiguousarray(t.reshape(PR, FW_QA))


def _shard_inputs(Q, I):
    Qp = np.pad(
        np.asarray(Q, np.float32), ((0, 0), (0, 0), (PAD, PAD), (PAD, PAD))
    ).astype(np.float16)
    Ip = np.pad(
        np.asarray(I, np.float32), ((0, 0), (0, 0), (PAD, PAD), (PAD, PAD))
    ).astype(np.float16)
    in_maps = []
    for b in range(B):
        for xs in range(4):
            c0 = xs * XSL
            in_maps.append(
                {
                    "i_in": _gather_i(Ip[b, :, :, c0 : c0 + 84]),
                    "q_in": _gather_q(Qp[b, :, :, c0 : c0 + 84]),
                }
            )
    return in_maps


def _assemble(outs):
    # outs: 8 arrays (128, 3600 = (c,q,n,x)), core order = (b, xs)
    o = np.stack([np.asarray(x) for x in outs]).astype(np.float32)
    o = o.reshape(B, 4, 2, 64, C, NQ, NCL, XWO)
    # (b, xs, xh, rr, c, q, n, x) -> (b, c, n, row=(q,rr), col=(xs,xh,x))
    o = o.transpose(0, 4, 6, 5, 3, 1, 2, 7).reshape(B, C, NCL, H, W)
    return o


def run(Q, I, trace=False):
    from concourse.bass_utils import run_bass_kernel_spmd

    nc = _get_program()
    in_maps = _shard_inputs(Q, I)
    res = run_bass_kernel_spmd(nc, in_maps, list(range(8)), trace=trace)
    out = _assemble(
        [
            np.concatenate(
                [res.results[i]["out_a"], res.results[i]["out_b"]], axis=1
            )
            for i in range(8)
        ]
    )
    return out, res


def kernel(Q, I):
    out, _ = run(Q, I)
    return out


# revision 23
# speedup vs baseline: 1.2647x; 1.0076x over previous
"""Bilateral filter (nn_BilateralFilter) Trainium2 Bass kernel.

Semantics (KERNEL_SIZE=5, THETA_ALPHA=2.0, THETA_BETA=0.1):
    w_k   = exp(-(dx^2+dy^2)/8)                      (24 offsets, center dropped)
    Ki    = exp(-50*(I(p+k) - I(p))^2)               per image channel c
    out[c,n,p] = sum_k w_k*Ki[c,k,p]*Q(n,p+k) / sum_k w_k*Ki[c,k,p]

Sharding: 8 cores = 2 batches x 4 col-slabs of 80 output cols.  Per core,
partitions = 128 = (xh in {0,1} col-half of 40) x (row mod 64); free dims =
(row-chunk q in [0,5), channel, x).

v3: every k-fold lives on the Tensor engine via identity-stationary matmuls
accumulating into PSUM:
  - acc  (sum_k kw*Q, 3600 fp32) in PSUM cols [0,3600), 512-col bank chunks
  - norm (sum_k kw) first 496 of its 600 cols in the PSUM bank-7 hole
    [3600,4096); the 104-col tail is a 6-op DVE tree (PSUM is 104 cols short)
DVE does subs, the 24 products, the tiny norm tail, reciprocal and the
final division (read straight from PSUM at 1x, split 1920+1680 so the first
output DMA overlaps the second division).  ACT does Square/Exp and the
fp32 stitch copies.  Outputs go to two contiguous DRAM tensors so the DMA
descriptors coalesce (a strided SBUF->HBM DMA is ~5x slower).
"""

import math

import numpy as np

B, C, NCL = 2, 3, 6
H = W = 320
KS, PAD = 5, 2
SHIFT = 8.0
COEF = 50.0
XSL = W // 4              # 80 output cols per core slab
XWO = 40                  # output cols per half
XWI = XWO + 2 * PAD       # 44 input cols per half
NQ = 5                    # row chunks of 64
PR = 128
HP = H + 2 * PAD          # 324 padded rows

IW = NQ * XWI             # 220   Ia per (s,c)
FW_IA = KS * C * IW       # 3300
QB = NCL * XWI            # 264   Qa per (s,c,q)
FW_QA = KS * C * NQ * QB  # 19800 (Q replicated x3 over c)
SLW = C * NQ * XWO        # 600   d/kw per slot (c,q,x)
NWX = NQ * NCL * XWO      # 1200  per-c product block (q,n,x)
CQN = C * NWX             # 3600  per-j product block (c,q,n,x)
FW_D = KS * KS * SLW      # 15000
FW_N = C * NQ * XWO       # 600   norm (c,q,x)

NPS = 496                 # norm cols accumulated in PSUM (bank-7 hole)
NTL = SLW - NPS           # 104   norm tail cols folded on DVE
CQ_A = 8                  # (c,q) blocks in the first div/DMA half
W_A = CQ_A * NCL * XWO    # 1920
W_B = CQN - W_A           # 1680

# PSUM bank = 512 fp32/partition; matmul output must stay inside one bank.
# Chunk 7 (bank 7, shared with the norm region) folds first per slot so the
# bank-7 accumulation group closes before the last slot's remaining folds
# and the norm reciprocal can overlap them.
MM_CHUNKS = [(3584, 3600)] + [(j * 512, (j + 1) * 512) for j in range(7)]

_CACHE: dict = {}


def _emit(tc, i_ap, q_ap, oa_ap, ob_ap):
    import concourse.bass as bass
    import concourse.mybir as mybir
    from concourse.masks import make_identity

    f16 = mybir.dt.float16
    f32 = mybir.dt.float32
    AF = mybir.ActivationFunctionType
    nc = tc.nc

    wy = [math.exp(-((s - PAD) ** 2) / 8.0) for s in range(KS)]

    def ap(t, off, dims):
        return bass.AP(tensor=t.tensor, offset=t.offset + off, ap=[[t.shape[1], PR]] + dims)

    with (
        tc.tile_pool(name="p", bufs=1) as pool,
        tc.tile_pool(name="p5p", bufs=4) as p5p,
        tc.tile_pool(name="ps", bufs=1, space="PSUM") as psp,
    ):
        Ia = pool.tile([PR, FW_IA], f16, tag="Ia")
        Qa = pool.tile([PR, FW_QA], f16, tag="Qa")
        d = pool.tile([PR, FW_D], f16, tag="d")
        kw = pool.tile([PR, FW_D], f16, tag="kw")
        otA = pool.tile([PR, W_A], f16, tag="otA")
        otB = pool.tile([PR, W_B], f16, tag="otB")

        normT = pool.tile([PR, 2 * NTL], f16, tag="normT")
        nt12 = pool.tile([PR, 12 * NTL], f16, tag="nt12")
        n32 = pool.tile([PR, NTL], f32, tag="n32")
        r32 = pool.tile([PR, FW_N], f32, tag="r32")

        ident = pool.tile([PR, PR], f16, tag="ident")
        acc = psp.tile([PR, 4096], f32, tag="acc")

        # per-slot exp biases SHIFT + ln(w_k) as const columns (5 distinct)
        bias_vals = sorted(
            {
                SHIFT + math.log(wy[s] * wy[dc])
                for s in range(KS)
                for dc in range(KS)
                if not (s == PAD and dc == PAD)
            }
        )
        bcol = {v: j for j, v in enumerate(bias_vals)}
        bias_t = pool.tile([PR, len(bias_vals)], f32, tag="bias")
        for v, j in bcol.items():
            nc.gpsimd.memset(bias_t[:, j : j + 1], v)

        # center slot of kw zeroed so the norm folds can include it blindly
        nc.gpsimd.memset(kw[:, 12 * SLW : 13 * SLW], 0.0)

        make_identity(nc, ident[:, :])

        # ---- input DMAs: host pre-gathers SBUF layouts; per-s slices on
        # separate queues so the early pipeline blocks land in parallel ----
        S_ORDER = [2, 0, 1, 3, 4]
        ia_q = {2: nc.sync, 0: nc.scalar, 1: nc.scalar, 3: nc.sync, 4: nc.gpsimd}
        for s in S_ORDER:
            ia_q[s].dma_start(
                Ia[:, s * C * IW : (s + 1) * C * IW],
                i_ap[:, s * C * IW : (s + 1) * C * IW],
            )
        QSW = C * NQ * QB  # 3960 per s
        for s in S_ORDER:
            nc.scalar.dma_start(
                Qa[:, s * QSW : (s + 1) * QSW],
                q_ap[:, s * QSW : (s + 1) * QSW],
            )

        # PE p-state warmup during the input-DMA window (bank-0 region is
        # reset by the real chunk-0 group's start=True later).
        for _ in range(12):
            nc.tensor.matmul(
                acc[:, 0:PR], ident[:, :], ident[:, :], start=True, stop=True
            )

        def sub_op(s, dc0, ndc):
            # d[(s,dc), (c,q), x] = Ia_s[(c,q), x+dc] - Ia_2[(c,q), x+2]
            nc.vector.tensor_sub(
                ap(d, (s * KS + dc0) * SLW, [[SLW, ndc], [XWO, C * NQ], [1, XWO]]),
                ap(Ia, s * C * IW + dc0, [[1, ndc], [XWI, C * NQ], [1, XWO]]),
                ap(Ia, 2 * C * IW + PAD, [[0, ndc], [XWI, C * NQ], [1, XWO]]),
            )

        n_slots = 0   # fold-group index over the 24 non-center slots
        n_norm = 0    # norm-group index over all 25 slots

        # Bank 7 ([3584,4096): acc chunk 7 + norm region) is ONE accumulation
        # group: start=True zeroes a whole 2KB bank, so the first bank-7 write
        # (norm_mm of the first slot) starts it and the last fold chunk-7
        # stops it.  Other banks group per-chunk as usual.
        def fold(p5t, idx):
            for c0, c1 in MM_CHUNKS:
                in_b7 = c0 >= 3584
                nc.tensor.matmul(
                    acc[:, c0:c1],
                    ident[:, :],
                    p5t[:, c0:c1],
                    start=(idx == 0) and not in_b7,
                    stop=(idx == 23),
                )

        def norm_mm(slot, idx):
            nc.tensor.matmul(
                acc[:, 3600 : 3600 + NPS],
                ident[:, :],
                kw[:, slot * SLW : slot * SLW + NPS],
                start=(idx == 0),
                stop=False,
            )

        # ---- per-s pipeline: sub -> square -> exp(+norm mm) -> product ->
        #      PE fold ----
        for si, s in enumerate(S_ORDER):
            if s == PAD:
                sub_op(s, 0, 2)
                sub_op(s, 3, 2)
                nc.scalar.activation(
                    kw[:, (s * KS) * SLW : (s * KS + 2) * SLW],
                    d[:, (s * KS) * SLW : (s * KS + 2) * SLW],
                    AF.Square,
                )
                nc.scalar.activation(
                    kw[:, (s * KS + 3) * SLW : (s * KS + 5) * SLW],
                    d[:, (s * KS + 3) * SLW : (s * KS + 5) * SLW],
                    AF.Square,
                )
            else:
                sub_op(s, 0, KS)
                nc.scalar.activation(
                    kw[:, (s * KS) * SLW : (s * KS + KS) * SLW],
                    d[:, (s * KS) * SLW : (s * KS + KS) * SLW],
                    AF.Square,
                )
            for dc in range(KS):
                slot = s * KS + dc
                if slot != 12:
                    j = bcol[SHIFT + math.log(wy[s] * wy[dc])]
                    nc.scalar.activation(
                        kw[:, slot * SLW : (slot + 1) * SLW],
                        kw[:, slot * SLW : (slot + 1) * SLW],
                        AF.Exp,
                        bias=bias_t[:, j : j + 1],
                        scale=-COEF,
                    )
                norm_mm(slot, n_norm)
                n_norm += 1

            def product(dc):
                slot = s * KS + dc
                p5t = p5p.tile([PR, CQN], f16, tag="p5")
                nc.vector.tensor_mul(
                    ap(p5t, 0, [[NWX // NQ, C * NQ], [XWO, NCL], [1, XWO]]),
                    ap(kw, slot * SLW, [[XWO, C * NQ], [0, NCL], [1, XWO]]),
                    ap(Qa, s * QSW + dc, [[QB, C * NQ], [XWI, NCL], [1, XWO]]),
                )
                return p5t

            dcs = [0, 1, 3, 4] if s == PAD else list(range(KS))
            for i, dc in enumerate(dcs):
                p5t = product(dc)
                fold(p5t, n_slots)
                n_slots += 1
                if si == 4 and i == 1:
                    # norm 104-col tail: DVE tree over all 25 slots
                    V = nc.vector
                    V.tensor_add(
                        nt12[:, :],
                        ap(kw, NPS, [[SLW, 12], [1, NTL]]),
                        ap(kw, 12 * SLW + NPS, [[SLW, 12], [1, NTL]]),
                    )
                    V.tensor_add(
                        nt12[:, : 6 * NTL], nt12[:, : 6 * NTL], nt12[:, 6 * NTL :]
                    )
                    V.tensor_add(
                        nt12[:, : 3 * NTL],
                        nt12[:, : 3 * NTL],
                        nt12[:, 3 * NTL : 6 * NTL],
                    )
                    V.tensor_add(
                        normT[:, :NTL], nt12[:, :NTL], nt12[:, NTL : 2 * NTL]
                    )
                    V.tensor_add(
                        normT[:, NTL:], normT[:, :NTL], nt12[:, 2 * NTL : 3 * NTL]
                    )
                    V.tensor_add(
                        normT[:, :NTL],
                        normT[:, NTL:],
                        kw[:, 24 * SLW + NPS : 25 * SLW],
                    )
                    nc.scalar.activation(n32[:, :], normT[:, :NTL], AF.Copy)
                elif si == 4 and i == 2:
                    nc.vector.reciprocal_approx_fast(r32[:, NPS:SLW], n32[:, :])

        # ---- tail: PSUM-direct reciprocal, split division + DMAs ----
        nc.vector.reciprocal_approx_fast(r32[:, :NPS], acc[:, 3600 : 3600 + NPS])

        CQ = C * NQ  # 15
        nc.vector.tensor_mul(
            ap(otA, 0, [[NCL * XWO, CQ_A], [XWO, NCL], [1, XWO]]),
            ap(acc, 0, [[NCL * XWO, CQ_A], [XWO, NCL], [1, XWO]]),
            ap(r32, 0, [[XWO, CQ_A], [0, NCL], [1, XWO]]),
        )
        nc.scalar.dma_start(oa_ap[:, :], otA[:, :])
        nc.vector.tensor_mul(
            ap(otB, 0, [[NCL * XWO, CQ - CQ_A], [XWO, NCL], [1, XWO]]),
            ap(acc, W_A, [[NCL * XWO, CQ - CQ_A], [XWO, NCL], [1, XWO]]),
            ap(r32, CQ_A * XWO, [[XWO, CQ - CQ_A], [0, NCL], [1, XWO]]),
        )
        nc.sync.dma_start(ob_ap[:, :], otB[:, :])


def _build_program():
    import concourse.bacc as bacc
    import concourse.mybir as mybir
    from concourse import tile

    f16 = mybir.dt.float16

    nc = bacc.Bacc("TRN2", num_devices=8, debug=False)
    I_in = nc.dram_tensor("i_in", [PR, FW_IA], f16, kind="ExternalInput")
    Q_in = nc.dram_tensor("q_in", [PR, FW_QA], f16, kind="ExternalInput")
    OUT_A = nc.dram_tensor("out_a", [PR, W_A], f16, kind="ExternalOutput")
    OUT_B = nc.dram_tensor("out_b", [PR, W_B], f16, kind="ExternalOutput")

    with tile.TileContext(nc) as tc:
        _emit(tc, I_in.ap(), Q_in.ap(), OUT_A.ap(), OUT_B.ap())

    nc.compile()
    return nc


def _get_program():
    if "nc" not in _CACHE:
        _CACHE["nc"] = _build_program()
    return _CACHE["nc"]


def _gather_i(Xp_sl):
    """(C, 324, 84) padded slab -> (128, (s,c,q,xi44)) fp16."""
    t = np.stack([Xp_sl[:, s : s + H, :] for s in range(KS)])  # (s,C,320,84)
    t = t.reshape(KS, C, NQ, 64, 84)
    t = np.stack([t[..., 40 * xh : 40 * xh + XWI] for xh in range(2)])
    # (xh, s, c, q, rr, xi) -> (xh, rr, s, c, q, xi)
    t = t.transpose(0, 4, 1, 2, 3, 5)
    return np.ascontiguousarray(t.reshape(PR, FW_IA))


def _gather_q(Qp_sl):
    """(NCL, 324, 84) padded slab -> (128, (s,c,q,n,xi44)) fp16, c-replicated."""
    t = np.stack([Qp_sl[:, s : s + H, :] for s in range(KS)])  # (s,NCL,320,84)
    t = t.reshape(KS, NCL, NQ, 64, 84)
    t = np.stack([t[..., 40 * xh : 40 * xh + XWI] for xh in range(2)])
    # (xh, s, n, q, rr, xi) -> (xh, rr, s, q, n, xi)
    t = t.transpose(0, 4, 1, 3, 2, 5)  # (xh, rr, s, q, n, xi)
    t = t.reshape(2, 64, KS, 1, NQ, NCL, XWI)
    t = np.broadcast_to(t, (2, 64, KS, C, NQ, NCL, XWI))
    return np.ascontiguousarray(t.reshape(PR, FW_QA))


def _shard_inputs(Q, I):
    Qp = np.pad(
        np.asarray(Q, np.float32), ((0, 0), (0, 0), (PAD, PAD), (PAD, PAD))
    ).astype(np.float16)
    Ip = np.pad(
        np.asarray(I, np.float32), ((0, 0), (0, 0), (PAD, PAD), (PAD, PAD))
    ).astype(np.float16)
    in_maps = []
    for b in range(B):
        for xs in range(4):
            c0 = xs * XSL
            in_maps.append(
                {
                    "i_in": _gather_i(Ip[b, :, :, c0 : c0 + 84]),
                    "q_in": _gather_q(Qp[b, :, :, c0 : c0 + 84]),
                }
            )
    return in_maps


def _assemble(outs):
    # outs: 8 arrays (128, 3600 = (c,q,n,x)), core order = (b, xs)
    o = np.stack([np.asarray(x) for x in outs]).astype(np.float32)
    o = o.reshape(B, 4, 2, 64, C, NQ, NCL, XWO)
    # (b, xs, xh, rr, c, q, n, x) -> (b, c, n, row=(q,rr), col=(xs,xh,x))
    o = o.transpose(0, 4, 6, 5, 3, 1, 2, 7).reshape(B, C, NCL, H, W)
    return o


def run(Q, I, trace=False):
    from concourse.bass_utils import run_bass_kernel_spmd

    nc = _get_program()
    in_maps = _shard_inputs(Q, I)
    res = run_bass_kernel_spmd(nc, in_maps, list(range(8)), trace=trace)
    out = _assemble(
        [
            np.concatenate(
                [res.results[i]["out_a"], res.results[i]["out_b"]], axis=1
            )
            for i in range(8)
        ]
    )
    return out, res


def kernel(Q, I):
    out, _ = run(Q, I)
    return out
